# revision 42
# baseline (speedup 1.0000x reference)
"""BiWindowMamba layer on 8 Trainium2 cores.

Sharding: core c = (dir, b, half) with dir=c//4, b=(c//2)%2, half=c%2.
Each core runs an IDENTICAL Bass program on different data:
  - backward-direction cores receive x[b] flipped in H and W (pooling and
    layernorm commute with the spatial flip, and flipping both pooled axes
    equals reversing the flattened L sequence), so every core runs a
    *forward* scan.
  - weights are permuted host-side so the core's local 256 d_inner channels
    occupy rows 0:256; the scan/dt/D/out_proj stages then address rows 0:256
    uniformly on every core.
Each core emits a partial (C=256, L=1024) out-projection (summed over its
256 local channels).  Host: flip L for backward partials, sum the 4 partials
per batch, reshape to (C,32,32), nearest-upsample x2, add residual.

Engine plan per core (constraints: GPSIMD may not touch PSUM and the
scan/divide ops exist only on DVE, so):
  - conv taps are diagonal-stationary PE matmuls accumulating in PSUM;
    silu via ACT Sigmoid (bias folds the conv bias) + DVE multiply.
  - layernorm stats via bf16 ones-matmuls; everything runs in bf16 so DVE
    TTs get the 2x perf mode and TSPs the 4x mode.
  - selective scan: ACT Exp for the 16 dA tiles ([128, 2048] md-combined),
    DVE tensor_tensor_scan, dbu/ymult TTs split across Pool/DVE, PE
    identity-matmul accumulation of y over the 16 states in PSUM, SP DMA
    partition-broadcasts of the B/C rows, with the front pipelined per
    512-column half (nh) and all softplus Lns deferred to minimize ACT
    table reloads.
"""

import ml_dtypes
import numpy as np

import concourse.bacc as bacc
import concourse.bass as bass  # noqa: F401
import concourse.mybir as mybir
import concourse.tile as tile

F32 = mybir.dt.float32
BF16 = mybir.dt.bfloat16
AF = mybir.ActivationFunctionType
OP = mybir.AluOpType

C = 256          # model dim
L = 1024         # tokens (32*32 pooled grid)
DF = 512         # full d_inner
DL = 256         # local d_inner shard
NS = 16          # d_state
RK = 16          # dt_rank
KC = 4           # d_conv
NCHIP = 8
BF16NP = ml_dtypes.bfloat16

# bf16 weight blob column layout
_WX = 0            # in_proj x-part, 2 k-blocks of [128, 512]
_WZ = 1024         # in_proj z-part, 2 k-blocks of [128, 256]
_XPW = 1536        # xproj_wT, 4 k-blocks of [128, 48]
_DTW = 1728        # dt_wT [16, 256] on partitions 0:16
_OW = 1984         # out_wT, 2 k-blocks of [128, 256]
_ID = 2496         # identity [128, 128]
_ONES1 = 2624      # [1, 128] ones on partition 0
_CD = 2752         # conv diag stationaries, (m, k) -> [128, 128]
_CBD = _CD + 16 * 128   # conv bias diag, m -> [128, 128]
_NB16 = _CBD + 4 * 128

# f32 weight blob column layout
_DTB = 0           # dt_b, 2 md-blocks of [128, 1]
_DP = 2            # D, 2 md-blocks
_G4 = 4            # 0.25 * ln_g, 2 c-blocks
_GN = 6            # -ln_g, 2 c-blocks
_BC = 8            # ln_b, 2 c-blocks
_AN = 10           # exp scales: col n = -(n+1), 16 cols
_CB = 26           # conv bias, 4 m-blocks
_CBN = 30          # negated conv bias, 4 m-blocks
_NF32 = 34

# 13 ACT exps; dA for n in {13,14,15} derived as products of earlier tiles
_PROD = {13: (6, 6), 14: (6, 7), 15: (7, 7)}


def build_nc():
    nc = bacc.Bacc("TRN2", target_bir_lowering=False, num_swdge_queues=4)

    xin = nc.dram_tensor("xin", [C, 64 * 64], F32, kind="ExternalInput")
    wb16 = nc.dram_tensor("wb16", [128, _NB16], BF16, kind="ExternalInput")
    wf32 = nc.dram_tensor("wf32", [128, _NF32], F32, kind="ExternalInput")
    dbl_d = nc.dram_tensor("dbl_d", [2 * NS, L], BF16, kind="Internal")
    part = nc.dram_tensor("part", [C, L], BF16, kind="ExternalOutput")

    with tile.TileContext(nc) as tc:
        with (
            tc.tile_pool(name="wpool", bufs=1) as wp,
            tc.tile_pool(name="act", bufs=1) as ap,
            tc.tile_pool(name="scan", bufs=2) as sp,
            tc.tile_pool(name="psum", bufs=1, space="PSUM") as pp,
        ):
            w16 = wp.tile([128, _NB16], BF16, tag="w16", name="w16")
            w32 = wp.tile([128, _NF32], F32, tag="w32", name="w32")

            def wx(k):
                return w16[:, _WX + k * 512:_WX + (k + 1) * 512]

            def wz(k):
                return w16[:, _WZ + k * 256:_WZ + (k + 1) * 256]

            def xpw(k):
                return w16[:, _XPW + k * 48:_XPW + (k + 1) * 48]

            def dtw(md):
                return w16[0:RK, _DTW + md * 128:_DTW + (md + 1) * 128]

            def ow(k):
                return w16[:, _OW + k * 256:_OW + (k + 1) * 256]

            ident = w16[:, _ID:_ID + 128]
            ones1 = w16[0:1, _ONES1:_ONES1 + 128]

            def cdiag(m, k):
                off = _CD + (m * 4 + k) * 128
                return w16[:, off:off + 128]

            def cbdiag(m):
                off = _CBD + m * 128
                return w16[:, off:off + 128]

            def dtb(md):
                return w32[:, _DTB + md:_DTB + md + 1]

            def dcol(md):
                return w32[:, _DP + md:_DP + md + 1]

            def g4(g):
                return w32[:, _G4 + g:_G4 + g + 1]

            def gneg(g):
                return w32[:, _GN + g:_GN + g + 1]

            def bcol(g):
                return w32[:, _BC + g:_BC + g + 1]

            def anscale(n):
                return w32[:, _AN + n:_AN + n + 1]

            def cbc(m):
                return w32[:, _CB + m:_CB + m + 1]

            def cbn(m):
                return w32[:, _CBN + m:_CBN + m + 1]

            # ---- input / weight DMAs across SP, ACT, Pool queues ----
            # x split into 8 pieces of 8 rows each: piece (g, j) covers
            # x rows 8j:8j+8 -> pooled rows 4j:4j+4 -> xp cols 128j:128j+128
            nc.scalar.dma_start(out=w32, in_=wf32[:, :])
            xt = {}
            order = [nc.sync, nc.scalar, nc.gpsimd, nc.sync,
                     nc.scalar, nc.gpsimd, nc.sync, nc.scalar]
            for g in range(2):
                for j in range(4):
                    t = ap.tile([128, 1024], F32, tag="xpiece", bufs=6,
                                name="xt")
                    src = xin[g * 128:(g + 1) * 128,
                              j * 1024:(j + 1) * 1024]
                    order[g * 4 + j].dma_start(out=t, in_=src)
                    xt[(g, j)] = t
            nc.sync.dma_start(out=w16[:, 0:_XPW], in_=wb16[:, 0:_XPW])
            nc.scalar.dma_start(out=w16[:, _XPW:_NB16],
                                in_=wb16[:, _XPW:_NB16])

            ones512 = wp.tile([128, 512], BF16, tag="ones512", name="ones512")
            nc.vector.memset(ones512, 1.0)
            onesb = wp.tile([128, 1], F32, tag="onesb", name="onesb")
            nc.vector.memset(onesb, 1.0)
            epsc = wp.tile([1, 1], F32, tag="epsc", name="epsc")
            nc.vector.memset(epsc, 1e-5)
            onecol = wp.tile([128, 1], BF16, tag="onecol", name="onecol")
            nc.vector.memset(onecol, 0.25 / C)
            onecol2 = wp.tile([128, 1], BF16, tag="onecol2", name="onecol2")
            nc.vector.memset(onecol2, 0.0625 / C)

            # ---- pooling: 2x2 avg kept as 4x sum (0.25 folded into LN) ----
            xp = []
            for g in range(2):
                xp_t = ap.tile([128, L], BF16, tag=f"xp{g}", name="xp_t")
                xp.append(xp_t)
            for g in range(2):
                for j in range(4):
                    # piece [128, 16 rows, 64w] -> pooled [128, 8, 32]
                    v = xt[(g, j)].rearrange(
                        "p (h hw w ww) -> p h hw w ww", hw=2, ww=2, w=32)
                    a1 = ap.tile([128, 8, 32], F32, tag="poolt", bufs=4,
                                 name="a1")
                    eng = nc.gpsimd
                    eng.tensor_add(out=a1, in0=v[:, :, 0, :, 0],
                                   in1=v[:, :, 0, :, 1])
                    eng.tensor_add(out=a1, in0=v[:, :, 1, :, 0], in1=a1)
                    eng.tensor_add(
                        out=xp[g][:, j * 256:(j + 1) * 256]
                        .rearrange("p (h w) -> p h w", w=32),
                        in0=v[:, :, 1, :, 1], in1=a1)

            # ---- layernorm + xn, pipelined per nh half-column block ----
            xsq = []
            for g in range(2):
                sq_t = ap.tile([128, L], BF16, tag=f"xsq{g}", name="sq_t")
                xsq.append(sq_t)
            rstd_b = ap.tile([128, L], BF16, tag="rstd_b", name="rstd_b")
            mr_b = ap.tile([128, L], BF16, tag="mr_b", name="mr_b")
            xn = [ap.tile([128, L], BF16, tag=f"xn{g}", name="xn_t")
                  for g in range(2)]
            for nh in range(2):
                sl = slice(nh * 512, (nh + 1) * 512)
                for g in range(2):
                    nc.gpsimd.tensor_mul(out=xsq[g][:, sl],
                                         in0=xp[g][:, sl], in1=xp[g][:, sl])
                mu_p = pp.tile([1, 512], F32, tag="mm", bufs=2, name="mu_p")
                ms_p = pp.tile([1, 512], F32, tag="mm", bufs=2, name="ms_p")
                for k in range(2):
                    nc.tensor.matmul(mu_p, onecol, xp[k][:, sl],
                                     start=(k == 0), stop=(k == 1))
                for k in range(2):
                    nc.tensor.matmul(ms_p, onecol2, xsq[k][:, sl],
                                     start=(k == 0), stop=(k == 1))
                mu = ap.tile([1, 512], BF16, tag="mu_sb", bufs=2, name="mu")
                ms = ap.tile([1, 512], BF16, tag="ms_sb", bufs=2, name="ms")
                nc.vector.tensor_copy(out=mu, in_=mu_p)
                nc.vector.tensor_copy(out=ms, in_=ms_p)
                mu2 = ap.tile([1, 512], BF16, tag="mu2", bufs=2, name="mu2")
                nc.gpsimd.tensor_mul(out=mu2, in0=mu, in1=mu)
                var = ap.tile([1, 512], BF16, tag="var", bufs=2, name="var")
                nc.gpsimd.tensor_sub(out=var, in0=ms, in1=mu2)
                rstd = ap.tile([1, 512], BF16, tag="rstd", bufs=2,
                               name="rstd")
                nc.scalar.activation(out=rstd, in_=var, func=AF.Sqrt,
                                     bias=epsc)
                with nc.allow_low_precision(reason="rstd ~2.0; bf16 ample"):
                    nc.vector.reciprocal(out=rstd, in_=rstd)
                mr = ap.tile([1, 512], BF16, tag="mr", bufs=2, name="mr")
                nc.gpsimd.tensor_mul(out=mr, in0=mu, in1=rstd)
                bp1 = pp.tile([128, 512], F32, tag="mm", bufs=2, name="bp1")
                nc.tensor.matmul(bp1, ones1, rstd, start=True, stop=True)
                bp2 = pp.tile([128, 512], F32, tag="mm", bufs=2, name="bp2")
                nc.tensor.matmul(bp2, ones1, mr, start=True, stop=True)
                nc.vector.tensor_copy(out=rstd_b[:, sl], in_=bp1)
                nc.vector.tensor_copy(out=mr_b[:, sl], in_=bp2)
                for g in range(2):
                    ct = ap.tile([128, 512], BF16, tag="ct", bufs=2,
                                 name="ct")
                    nc.vector.tensor_scalar(
                        out=ct, in0=mr_b[:, sl], scalar1=gneg(g),
                        scalar2=bcol(g), op0=OP.mult, op1=OP.add)
                    nc.vector.tensor_scalar_mul(out=xn[g][:, sl],
                                                in0=xp[g][:, sl],
                                                scalar1=g4(g))
                    nc.gpsimd.tensor_mul(out=xn[g][:, sl],
                                         in0=xn[g][:, sl],
                                         in1=rstd_b[:, sl])
                    nc.gpsimd.tensor_add(out=xn[g][:, sl],
                                         in0=xn[g][:, sl], in1=ct)

            # ---- in_proj -> conv -> silu -> xproj -> dt, nh-major ----
            xct = [ap.tile([128, L], BF16, tag=f"xc{m}", name="xc_t")
                   for m in range(4)]
            ut = [ap.tile([128, L], BF16, tag=f"u{m}", name="u_t")
                  for m in range(4)]
            dtm = ap.tile([RK, L], BF16, tag="dtm", name="dtm")
            dblh = ap.tile([2 * NS, L], BF16, tag="dblh", name="dblh")
            delta = ap.tile([128, 2 * L], BF16, tag="delta", name="delta")
            for nh in range(2):
                lo, hi = nh * 512, (nh + 1) * 512
                sl = slice(lo, hi)
                ps_bc = pp.tile([2 * NS, 512], F32, tag="xps", bufs=2,
                                name="ps_bc")
                ps_dt = pp.tile([RK, 512], F32, tag="xps", bufs=2,
                                name="ps_dt")
                for m in range(4):
                    ps = pp.tile([128, 512], F32, tag="mm", bufs=2,
                                 name="ps")
                    for k in range(2):
                        nc.tensor.matmul(
                            ps, wx(k)[:, m * 128:(m + 1) * 128],
                            xn[k][:, sl], start=(k == 0), stop=(k == 1))
                    nc.scalar.activation(out=xct[m][:, sl], in_=ps,
                                          func=AF.Copy)
                    cps = pp.tile([128, 512], F32, tag="mm", bufs=2,
                                  name="cps")
                    for k in (3, 2, 1, 0):
                        s = 3 - k
                        olo = max(lo, s)
                        nc.tensor.matmul(
                            cps[:, olo - lo:512], cdiag(m, k),
                            xct[m][:, olo - s:hi - s],
                            start=(k == 3), stop=(k == 0))
                    sg = ap.tile([128, 512], BF16, tag="sg", bufs=4,
                                 name="sg")
                    nc.scalar.activation(out=sg, in_=cps, func=AF.Sigmoid,
                                         bias=cbc(m))
                    num = ap.tile([128, 512], BF16, tag="num", bufs=4,
                                  name="num")
                    if m % 2 == 0:
                        nc.scalar.activation(out=num, in_=cps,
                                             func=AF.Identity, bias=cbc(m))
                    else:
                        nc.vector.tensor_scalar(out=num, in0=cps,
                                                scalar1=1.0, scalar2=cbc(m),
                                                op0=OP.mult, op1=OP.add)
                    nc.gpsimd.tensor_mul(out=ut[m][:, sl], in0=num, in1=sg)
                    nc.tensor.matmul(ps_bc, xpw(m)[:, RK:RK + 2 * NS],
                                     ut[m][:, sl],
                                     start=(m == 0), stop=(m == 3))
                    nc.tensor.matmul(ps_dt, xpw(m)[:, 0:RK],
                                     ut[m][:, sl],
                                     start=(m == 0), stop=(m == 3))
                nc.vector.tensor_copy(out=dblh[:, sl], in_=ps_bc)
                nc.vector.tensor_copy(out=dtm[:, sl], in_=ps_dt)
                # dt-proj + softplus exp for this half (Ln deferred so all
                # front Exps share one act table residency)
                for md in range(2):
                    dps = pp.tile([128, 512], F32, tag="mm", bufs=2,
                                  name="dps")
                    nc.tensor.matmul(dps, dtw(md), dtm[:, sl],
                                     start=True, stop=True)
                    dlo = md * L + nh * 512
                    dsl = delta[:, dlo:dlo + 512]
                    nc.scalar.activation(out=dsl, in_=dps, func=AF.Exp,
                                         bias=dtb(md))
            nc.sync.dma_start(out=dbl_d[:, :], in_=dblh)

            # B/C partition-broadcasts: emit right away so the SP queue
            # streams them while dt/softplus still computes
            bbt, cbt = [], []
            for n in range(NS):
                bbn = sp.tile([128, L], BF16, tag="bb", bufs=5, name="bb")
                nc.sync.dma_start(
                    out=bbn, in_=dbl_d[n:n + 1, :].to_broadcast([128, L]))
                cbn = sp.tile([128, L], BF16, tag="cbr", bufs=5, name="cbr")
                nc.sync.dma_start(
                    out=cbn,
                    in_=dbl_d[NS + n:NS + n + 1, :].to_broadcast([128, L]))
                bbt.append(bbn)
                cbt.append(cbn)

            dview = delta.rearrange("p (c t) -> p c t", c=2)

            # ---- z part: sz = z / (1+exp(-z)) ----
            sz = []
            for mz in range(2):
                z_t = ap.tile([128, L], BF16, tag=f"z{mz}", name="z_t")
                sz_t = ap.tile([128, L], BF16, tag=f"sz{mz}", name="sz_t")
                for nh in range(2):
                    sl = slice(nh * 512, (nh + 1) * 512)
                    ps = pp.tile([128, 512], F32, tag="mm", bufs=2,
                                 name="zps")
                    for k in range(2):
                        nc.tensor.matmul(
                            ps, wz(k)[:, mz * 128:(mz + 1) * 128],
                            xn[k][:, sl], start=(k == 0), stop=(k == 1))
                    nc.vector.tensor_scalar_mul(out=z_t[:, sl], in0=ps,
                                                scalar1=1.0)
                    sg = ap.tile([128, 512], BF16, tag="sg", bufs=4,
                                 name="sg")
                    nc.scalar.activation(out=sg, in_=ps, func=AF.Sigmoid)
                    nc.gpsimd.tensor_mul(out=sz_t[:, sl],
                                          in0=z_t[:, sl], in1=sg)
                sz.append(sz_t)


            # deferred softplus Ln: delta = ln(1 + e^x), per md half
            for md in range(2):
                dsl = delta[:, md * L:(md + 1) * L]
                nc.scalar.activation(out=dsl, in_=dsl, func=AF.Ln,
                                     bias=onesb)

            # ---- du = delta * u ; ud = u * D ----
            du = ap.tile([128, 2 * L], BF16, tag="du", name="du")
            duv = du.rearrange("p (c t) -> p c t", c=2)
            ud = []
            for md in range(2):
                nc.gpsimd.tensor_mul(out=duv[:, md], in0=dview[:, md],
                                      in1=ut[md])
                ud_t = ap.tile([128, L], BF16, tag=f"ud{md}", name="ud_t")
                nc.gpsimd.tensor_scalar_mul(out=ud_t, in0=ut[md],
                                            scalar1=dcol(md))
                ud.append(ud_t)

            # ---- selective scan; y accumulated in PSUM via PE ----
            yacc = [pp.tile([128, L], F32, tag=f"yacc{md}", name="yacc")
                    for md in range(2)]
            da_keep = {}
            for n in range(NS):
                if n in (6, 7):
                    da = sp.tile([128, 2 * L], BF16, tag=f"dakeep{n}",
                                 bufs=1, name="da")
                else:
                    da = sp.tile([128, 2 * L], BF16, tag="da", bufs=3,
                                 name="da")
                if n in _PROD:
                    ja, jb = _PROD[n]
                    nc.vector.tensor_mul(out=da, in0=da_keep[ja],
                                         in1=da_keep[jb])
                else:
                    nc.scalar.activation(out=da, in_=delta, func=AF.Exp,
                                         scale=anscale(n))
                if n in (6, 7):
                    da_keep[n] = da
                dav = da.rearrange("p (c t) -> p c t", c=2)

                dbu = sp.tile([128, 2 * L], BF16, tag="dbu", bufs=3,
                              name="dbu")
                dbuv = dbu.rearrange("p (c t) -> p c t", c=2)
                for md in range(2):
                    nc.gpsimd.tensor_mul(out=dbuv[:, md], in0=duv[:, md],
                                         in1=bbt[n])

                h = sp.tile([128, 2 * L], BF16, tag="h", bufs=3, name="h")
                hv = h.rearrange("p (c t) -> p c t", c=2)
                for md in range(2):
                    nc.vector.tensor_tensor_scan(
                        out=hv[:, md], data0=dav[:, md], data1=dbuv[:, md],
                        initial=0.0, op0=OP.mult, op1=OP.add)

                yt = sp.tile([128, 2 * L], BF16, tag="yt", bufs=2, name="yt")
                ytv = yt.rearrange("p (c t) -> p c t", c=2)
                eng0 = nc.gpsimd if n in (3, 7, 11, 15) else nc.vector
                eng0.tensor_mul(out=ytv[:, 0], in0=hv[:, 0], in1=cbt[n])
                nc.gpsimd.tensor_mul(out=ytv[:, 1], in0=hv[:, 1],
                                     in1=cbt[n])
                for md in range(2):
                    for q in range(2):
                        sl = slice(q * 512, (q + 1) * 512)
                        nc.tensor.matmul(
                            yacc[md][:, sl], ident, ytv[:, md][:, sl],
                            start=(n == 0), stop=(n == NS - 1))

            # ---- tail: y = (yacc + u*D) * silu(z); partial out-proj ----
            yf = []
            for md in range(2):
                y1 = ap.tile([128, L], BF16, tag=f"y1{md}", name="y1")
                nc.vector.tensor_add(out=y1, in0=yacc[md], in1=ud[md])
                yf_t = ap.tile([128, L], BF16, tag=f"yf{md}", name="yf_t")
                nc.gpsimd.tensor_mul(out=yf_t, in0=y1, in1=sz[md])
                yf.append(yf_t)
            for mc in range(2):
                pt = ap.tile([128, L], BF16, tag="part", bufs=2, name="pt")
                for nh in range(2):
                    sl = slice(nh * 512, (nh + 1) * 512)
                    po = pp.tile([128, 512], F32, tag="mm", bufs=2,
                                 name="po")
                    for k in range(2):
                        nc.tensor.matmul(
                            po, ow(k)[:, mc * 128:(mc + 1) * 128],
                            yf[k][:, sl], start=(k == 0), stop=(k == 1))
                    nc.scalar.activation(out=pt[:, sl], in_=po,
                                         func=AF.Copy)
                nc.sync.dma_start(
                    out=part[mc * 128:(mc + 1) * 128, :], in_=pt)
    nc.compile()
    return nc


def make_in_maps(inputs):
    x = np.asarray(inputs["x"], np.float32)
    maps = []
    for c in range(NCHIP):
        dr, b, half = c // 4, (c // 2) % 2, c % 2
        p = "f_" if dr == 0 else "b_"
        in_w = np.asarray(inputs[p + "in_w"], np.float32)
        convw = np.asarray(inputs[p + "conv_w"], np.float32)
        convb = np.asarray(inputs[p + "conv_b"], np.float32)
        xpj = np.asarray(inputs[p + "xproj_w"], np.float32)
        dtw = np.asarray(inputs[p + "dt_w"], np.float32)
        dtb = np.asarray(inputs[p + "dt_b"], np.float32)
        dpar = np.asarray(inputs[p + "D"], np.float32)
        outw = np.asarray(inputs["out_w"], np.float32)
        ln_g = np.asarray(inputs["ln_g"], np.float32)
        ln_b = np.asarray(inputs["ln_b"], np.float32)

        px = np.concatenate([np.arange(DL) + half * DL,
                             np.arange(DL) + (1 - half) * DL])
        loc = px[:DL]
        xin = x[b] if dr == 0 else x[b, :, ::-1, ::-1]

        w16 = np.zeros((128, _NB16), np.float32)
        wxT = in_w[:DF][px].T          # [C, DF]
        w16[:, _WX:_WX + 512] = wxT[0:128]
        w16[:, _WX + 512:_WX + 1024] = wxT[128:256]
        wzT = in_w[DF:][loc].T         # [C, DL]
        w16[:, _WZ:_WZ + 256] = wzT[0:128]
        w16[:, _WZ + 256:_WZ + 512] = wzT[128:256]
        xpwT = xpj[:, px].T            # [DF, 48]
        for k in range(4):
            w16[:, _XPW + k * 48:_XPW + (k + 1) * 48] = \
                xpwT[k * 128:(k + 1) * 128]
        w16[0:RK, _DTW:_DTW + 256] = dtw[loc].T
        owT = outw[:, loc].T           # [DL, C]
        w16[:, _OW:_OW + 256] = owT[0:128]
        w16[:, _OW + 256:_OW + 512] = owT[128:256]
        w16[:, _ID:_ID + 128] = np.eye(128, dtype=np.float32)
        w16[0, _ONES1:_ONES1 + 128] = 1.0
        cwl = convw[:, 0, :][px]       # [DF, 4]
        cbl = convb[px]
        for m in range(4):
            for k in range(4):
                off = _CD + (m * 4 + k) * 128
                w16[:, off:off + 128] = np.diag(
                    cwl[m * 128:(m + 1) * 128, k])
            off = _CBD + m * 128
            w16[:, off:off + 128] = np.diag(cbl[m * 128:(m + 1) * 128])

        w32 = np.zeros((128, _NF32), np.float32)
        for md in range(2):
            w32[:, _DTB + md] = dtb[loc][md * 128:(md + 1) * 128]
            w32[:, _DP + md] = dpar[loc][md * 128:(md + 1) * 128]
        for g in range(2):
            w32[:, _G4 + g] = 0.25 * ln_g[g * 128:(g + 1) * 128]
            w32[:, _GN + g] = -ln_g[g * 128:(g + 1) * 128]
            w32[:, _BC + g] = ln_b[g * 128:(g + 1) * 128]
        for n in range(NS):
            w32[:, _AN + n] = -float(n + 1)
        for m in range(4):
            w32[:, _CB + m] = cbl[m * 128:(m + 1) * 128]
            w32[:, _CBN + m] = -cbl[m * 128:(m + 1) * 128]

        maps.append({
            "xin": np.ascontiguousarray(xin.reshape(C, 64 * 64)),
            "wb16": w16.astype(BF16NP),
            "wf32": np.ascontiguousarray(w32),
        })
    return maps


def combine(parts, x):
    out = np.empty_like(x)
    for b in range(2):
        acc = np.zeros((C, L), np.float32)
        for c in range(NCHIP):
            dr, bb, _ = c // 4, (c // 2) % 2, c % 2
            if bb != b:
                continue
            pc = np.asarray(parts[c], np.float32)
            if dr == 1:
                pc = pc[:, ::-1]
            acc += pc
        o = acc.reshape(C, 32, 32)
        o = np.repeat(np.repeat(o, 2, axis=1), 2, axis=2)
        out[b] = o + x[b]
    return out


_NC_CACHE = None


def _get_nc():
    global _NC_CACHE
    if _NC_CACHE is None:
        _NC_CACHE = build_nc()
    return _NC_CACHE


def kernel(**inputs):
    from concourse.bass_utils import run_bass_kernel_spmd

    nc = _get_nc()
    in_maps = make_in_maps(inputs)
    res = run_bass_kernel_spmd(nc, in_maps, core_ids=list(range(NCHIP)))
    parts = [r["part"] for r in res.results]
    return combine(parts, np.asarray(inputs["x"], np.float32))


# revision 45
# speedup vs baseline: 1.0289x; 1.0289x over previous
"""BiWindowMamba layer on 8 Trainium2 cores.

Sharding: core c = (dir, b, half) with dir=c//4, b=(c//2)%2, half=c%2.
Each core runs an IDENTICAL Bass program on different data:
  - backward-direction cores receive x[b] flipped in H and W (pooling and
    layernorm commute with the spatial flip, and flipping both pooled axes
    equals reversing the flattened L sequence), so every core runs a
    *forward* scan.
  - weights are permuted host-side so the core's local 256 d_inner channels
    occupy rows 0:256; the scan/dt/D/out_proj stages then address rows 0:256
    uniformly on every core.
Each core emits a partial (C=256, L=1024) out-projection (summed over its
256 local channels).  Host: flip L for backward partials, sum the 4 partials
per batch, reshape to (C,32,32), nearest-upsample x2, add residual.

Engine plan per core (constraints: GPSIMD may not touch PSUM and the
scan/divide ops exist only on DVE, so):
  - conv taps are diagonal-stationary PE matmuls accumulating in PSUM;
    silu via ACT Sigmoid (bias folds the conv bias) + DVE multiply.
  - layernorm stats via bf16 ones-matmuls; everything runs in bf16 so DVE
    TTs get the 2x perf mode and TSPs the 4x mode.
  - selective scan: ACT Exp for the 16 dA tiles ([128, 2048] md-combined),
    DVE tensor_tensor_scan, dbu/ymult TTs split across Pool/DVE, PE
    identity-matmul accumulation of y over the 16 states in PSUM, SP DMA
    partition-broadcasts of the B/C rows, with the front pipelined per
    512-column half (nh) and all softplus Lns deferred to minimize ACT
    table reloads.
"""

import ml_dtypes
import numpy as np

import concourse.bacc as bacc
import concourse.bass as bass  # noqa: F401
import concourse.mybir as mybir
import concourse.tile as tile

F32 = mybir.dt.float32
BF16 = mybir.dt.bfloat16
AF = mybir.ActivationFunctionType
OP = mybir.AluOpType

C = 256          # model dim
L = 1024         # tokens (32*32 pooled grid)
DF = 512         # full d_inner
DL = 256         # local d_inner shard
NS = 16          # d_state
RK = 16          # dt_rank
KC = 4           # d_conv
NCHIP = 8
BF16NP = ml_dtypes.bfloat16

# bf16 weight blob column layout
_WX = 0            # in_proj x-part, 2 k-blocks of [128, 512]
_WZ = 1024         # in_proj z-part, 2 k-blocks of [128, 256]
_XPW = 1536        # xproj_wT, 4 k-blocks of [128, 48]
_DTW = 1728        # dt_wT [16, 256] on partitions 0:16
_OW = 1984         # out_wT, 2 k-blocks of [128, 256]
_ID = 2496         # identity [128, 128]
_ONES1 = 2624      # [1, 128] ones on partition 0
_CD = 2752         # conv diag stationaries, (m, k) -> [128, 128]
_CBD = _CD + 16 * 128   # conv bias diag, m -> [128, 128]
_NB16 = _CBD + 4 * 128

# f32 weight blob column layout
_DTB = 0           # dt_b, 2 md-blocks of [128, 1]
_DP = 2            # D, 2 md-blocks
_G4 = 4            # 0.25 * ln_g, 2 c-blocks
_GN = 6            # -ln_g, 2 c-blocks
_BC = 8            # ln_b, 2 c-blocks
_AN = 10           # exp scales: col n = -(n+1), 16 cols
_CB = 26           # conv bias, 4 m-blocks
_CBN = 30          # negated conv bias, 4 m-blocks
_NF32 = 34

# all 16 dA tiles as ACT exps: ACT has loop slack, while TT products would
# land on DVE right in the loop drain where it is the bottleneck
_PROD = {}


def build_nc():
    nc = bacc.Bacc("TRN2", target_bir_lowering=False, num_swdge_queues=4)

    xin = nc.dram_tensor("xin", [C, 64 * 64], F32, kind="ExternalInput")
    wb16 = nc.dram_tensor("wb16", [128, _NB16], BF16, kind="ExternalInput")
    wf32 = nc.dram_tensor("wf32", [128, _NF32], F32, kind="ExternalInput")
    dbl_d = nc.dram_tensor("dbl_d", [2 * NS, L], BF16, kind="Internal")
    part = nc.dram_tensor("part", [C, L], BF16, kind="ExternalOutput")

    with tile.TileContext(nc) as tc:
        with (
            tc.tile_pool(name="wpool", bufs=1) as wp,
            tc.tile_pool(name="act", bufs=1) as ap,
            tc.tile_pool(name="scan", bufs=2) as sp,
            tc.tile_pool(name="psum", bufs=1, space="PSUM") as pp,
        ):
            w16 = wp.tile([128, _NB16], BF16, tag="w16", name="w16")
            w32 = wp.tile([128, _NF32], F32, tag="w32", name="w32")

            def wx(k):
                return w16[:, _WX + k * 512:_WX + (k + 1) * 512]

            def wz(k):
                return w16[:, _WZ + k * 256:_WZ + (k + 1) * 256]

            def xpw(k):
                return w16[:, _XPW + k * 48:_XPW + (k + 1) * 48]

            def dtw(md):
                return w16[0:RK, _DTW + md * 128:_DTW + (md + 1) * 128]

            def ow(k):
                return w16[:, _OW + k * 256:_OW + (k + 1) * 256]

            ident = w16[:, _ID:_ID + 128]
            ones1 = w16[0:1, _ONES1:_ONES1 + 128]

            def cdiag(m, k):
                off = _CD + (m * 4 + k) * 128
                return w16[:, off:off + 128]

            def cbdiag(m):
                off = _CBD + m * 128
                return w16[:, off:off + 128]

            def dtb(md):
                return w32[:, _DTB + md:_DTB + md + 1]

            def dcol(md):
                return w32[:, _DP + md:_DP + md + 1]

            def g4(g):
                return w32[:, _G4 + g:_G4 + g + 1]

            def gneg(g):
                return w32[:, _GN + g:_GN + g + 1]

            def bcol(g):
                return w32[:, _BC + g:_BC + g + 1]

            def anscale(n):
                return w32[:, _AN + n:_AN + n + 1]

            def cbc(m):
                return w32[:, _CB + m:_CB + m + 1]

            def cbn(m):
                return w32[:, _CBN + m:_CBN + m + 1]

            # ---- input / weight DMAs across SP, ACT, Pool queues ----
            # x split into 8 pieces of 8 rows each: piece (g, j) covers
            # x rows 8j:8j+8 -> pooled rows 4j:4j+4 -> xp cols 128j:128j+128
            nc.scalar.dma_start(out=w32, in_=wf32[:, :])
            xt = {}
            order = [nc.sync, nc.scalar, nc.gpsimd, nc.sync,
                     nc.scalar, nc.gpsimd, nc.sync, nc.scalar]
            for g in range(2):
                for j in range(4):
                    t = ap.tile([128, 1024], F32, tag="xpiece", bufs=6,
                                name="xt")
                    src = xin[g * 128:(g + 1) * 128,
                              j * 1024:(j + 1) * 1024]
                    order[g * 4 + j].dma_start(out=t, in_=src)
                    xt[(g, j)] = t
            nc.sync.dma_start(out=w16[:, 0:_XPW], in_=wb16[:, 0:_XPW])
            nc.scalar.dma_start(out=w16[:, _XPW:_NB16],
                                in_=wb16[:, _XPW:_NB16])

            ones512 = wp.tile([128, 512], BF16, tag="ones512", name="ones512")
            nc.vector.memset(ones512, 1.0)
            onesb = wp.tile([128, 1], F32, tag="onesb", name="onesb")
            nc.vector.memset(onesb, 1.0)
            epsc = wp.tile([1, 1], F32, tag="epsc", name="epsc")
            nc.vector.memset(epsc, 1e-5)
            onecol = wp.tile([128, 1], BF16, tag="onecol", name="onecol")
            nc.vector.memset(onecol, 0.25 / C)
            onecol2 = wp.tile([128, 1], BF16, tag="onecol2", name="onecol2")
            nc.vector.memset(onecol2, 0.0625 / C)

            # ---- pooling: 2x2 avg kept as 4x sum (0.25 folded into LN) ----
            xp = []
            for g in range(2):
                xp_t = ap.tile([128, L], BF16, tag=f"xp{g}", name="xp_t")
                xp.append(xp_t)
            for g in range(2):
                for j in range(4):
                    # piece [128, 16 rows, 64w] -> pooled [128, 8, 32]
                    v = xt[(g, j)].rearrange(
                        "p (h hw w ww) -> p h hw w ww", hw=2, ww=2, w=32)
                    a1 = ap.tile([128, 8, 32], F32, tag="poolt", bufs=4,
                                 name="a1")
                    eng = nc.gpsimd
                    eng.tensor_add(out=a1, in0=v[:, :, 0, :, 0],
                                   in1=v[:, :, 0, :, 1])
                    eng.tensor_add(out=a1, in0=v[:, :, 1, :, 0], in1=a1)
                    eng.tensor_add(
                        out=xp[g][:, j * 256:(j + 1) * 256]
                        .rearrange("p (h w) -> p h w", w=32),
                        in0=v[:, :, 1, :, 1], in1=a1)

            # ---- layernorm + xn, pipelined per nh half-column block ----
            xsq = []
            for g in range(2):
                sq_t = ap.tile([128, L], BF16, tag=f"xsq{g}", name="sq_t")
                xsq.append(sq_t)
            rstd_b = ap.tile([128, L], BF16, tag="rstd_b", name="rstd_b")
            mr_b = ap.tile([128, L], BF16, tag="mr_b", name="mr_b")
            xn = [ap.tile([128, L], BF16, tag=f"xn{g}", name="xn_t")
                  for g in range(2)]
            for nh in range(2):
                sl = slice(nh * 512, (nh + 1) * 512)
                for g in range(2):
                    nc.gpsimd.tensor_mul(out=xsq[g][:, sl],
                                         in0=xp[g][:, sl], in1=xp[g][:, sl])
                mu_p = pp.tile([1, 512], F32, tag="mm", bufs=2, name="mu_p")
                ms_p = pp.tile([1, 512], F32, tag="mm", bufs=2, name="ms_p")
                for k in range(2):
                    nc.tensor.matmul(mu_p, onecol, xp[k][:, sl],
                                     start=(k == 0), stop=(k == 1))
                for k in range(2):
                    nc.tensor.matmul(ms_p, onecol2, xsq[k][:, sl],
                                     start=(k == 0), stop=(k == 1))
                mu = ap.tile([1, 512], BF16, tag="mu_sb", bufs=2, name="mu")
                ms = ap.tile([1, 512], BF16, tag="ms_sb", bufs=2, name="ms")
                nc.vector.tensor_copy(out=mu, in_=mu_p)
                nc.vector.tensor_copy(out=ms, in_=ms_p)
                mu2 = ap.tile([1, 512], BF16, tag="mu2", bufs=2, name="mu2")
                nc.gpsimd.tensor_mul(out=mu2, in0=mu, in1=mu)
                var = ap.tile([1, 512], BF16, tag="var", bufs=2, name="var")
                nc.gpsimd.tensor_sub(out=var, in0=ms, in1=mu2)
                rstd = ap.tile([1, 512], BF16, tag="rstd", bufs=2,
                               name="rstd")
                nc.scalar.activation(out=rstd, in_=var, func=AF.Sqrt,
                                     bias=epsc)
                with nc.allow_low_precision(reason="rstd ~2.0; bf16 ample"):
                    nc.vector.reciprocal(out=rstd, in_=rstd)
                mr = ap.tile([1, 512], BF16, tag="mr", bufs=2, name="mr")
                nc.gpsimd.tensor_mul(out=mr, in0=mu, in1=rstd)
                bp1 = pp.tile([128, 512], F32, tag="mm", bufs=2, name="bp1")
                nc.tensor.matmul(bp1, ones1, rstd, start=True, stop=True)
                bp2 = pp.tile([128, 512], F32, tag="mm", bufs=2, name="bp2")
                nc.tensor.matmul(bp2, ones1, mr, start=True, stop=True)
                nc.vector.tensor_copy(out=rstd_b[:, sl], in_=bp1)
                nc.vector.tensor_copy(out=mr_b[:, sl], in_=bp2)
                for g in range(2):
                    ct = ap.tile([128, 512], BF16, tag="ct", bufs=2,
                                 name="ct")
                    nc.vector.tensor_scalar(
                        out=ct, in0=mr_b[:, sl], scalar1=gneg(g),
                        scalar2=bcol(g), op0=OP.mult, op1=OP.add)
                    nc.vector.tensor_scalar_mul(out=xn[g][:, sl],
                                                in0=xp[g][:, sl],
                                                scalar1=g4(g))
                    nc.gpsimd.tensor_mul(out=xn[g][:, sl],
                                         in0=xn[g][:, sl],
                                         in1=rstd_b[:, sl])
                    nc.gpsimd.tensor_add(out=xn[g][:, sl],
                                         in0=xn[g][:, sl], in1=ct)

            # ---- in_proj -> conv -> silu -> xproj -> dt, nh-major ----
            xct = [ap.tile([128, L], BF16, tag=f"xc{m}", name="xc_t")
                   for m in range(4)]
            ut = [ap.tile([128, L], BF16, tag=f"u{m}", name="u_t")
                  for m in range(4)]
            dtm = ap.tile([RK, L], BF16, tag="dtm", name="dtm")
            dblh = ap.tile([2 * NS, L], BF16, tag="dblh", name="dblh")
            delta = ap.tile([128, 2 * L], BF16, tag="delta", name="delta")
            for nh in range(2):
                lo, hi = nh * 512, (nh + 1) * 512
                sl = slice(lo, hi)
                ps_bc = pp.tile([2 * NS, 512], F32, tag="xps", bufs=2,
                                name="ps_bc")
                ps_dt = pp.tile([RK, 512], F32, tag="xps", bufs=2,
                                name="ps_dt")
                for m in range(4):
                    ps = pp.tile([128, 512], F32, tag="mm", bufs=2,
                                 name="ps")
                    for k in range(2):
                        nc.tensor.matmul(
                            ps, wx(k)[:, m * 128:(m + 1) * 128],
                            xn[k][:, sl], start=(k == 0), stop=(k == 1))
                    nc.scalar.activation(out=xct[m][:, sl], in_=ps,
                                          func=AF.Copy)
                    cps = pp.tile([128, 512], F32, tag="mm", bufs=2,
                                  name="cps")
                    for k in (3, 2, 1, 0):
                        s = 3 - k
                        olo = max(lo, s)
                        nc.tensor.matmul(
                            cps[:, olo - lo:512], cdiag(m, k),
                            xct[m][:, olo - s:hi - s],
                            start=(k == 3), stop=(k == 0))
                    sg = ap.tile([128, 512], BF16, tag="sg", bufs=4,
                                 name="sg")
                    nc.scalar.activation(out=sg, in_=cps, func=AF.Sigmoid,
                                         bias=cbc(m))
                    num = ap.tile([128, 512], BF16, tag="num", bufs=4,
                                  name="num")
                    if m % 2 == 0:
                        nc.scalar.activation(out=num, in_=cps,
                                             func=AF.Identity, bias=cbc(m))
                    else:
                        nc.vector.tensor_scalar(out=num, in0=cps,
                                                scalar1=1.0, scalar2=cbc(m),
                                                op0=OP.mult, op1=OP.add)
                    nc.gpsimd.tensor_mul(out=ut[m][:, sl], in0=num, in1=sg)
                    nc.tensor.matmul(ps_bc, xpw(m)[:, RK:RK + 2 * NS],
                                     ut[m][:, sl],
                                     start=(m == 0), stop=(m == 3))
                    nc.tensor.matmul(ps_dt, xpw(m)[:, 0:RK],
                                     ut[m][:, sl],
                                     start=(m == 0), stop=(m == 3))
                nc.vector.tensor_copy(out=dblh[:, sl], in_=ps_bc)
                nc.vector.tensor_copy(out=dtm[:, sl], in_=ps_dt)
                # dt-proj + softplus exp for this half (Ln deferred so all
                # front Exps share one act table residency)
                for md in range(2):
                    dps = pp.tile([128, 512], F32, tag="mm", bufs=2,
                                  name="dps")
                    nc.tensor.matmul(dps, dtw(md), dtm[:, sl],
                                     start=True, stop=True)
                    dlo = md * L + nh * 512
                    dsl = delta[:, dlo:dlo + 512]
                    nc.scalar.activation(out=dsl, in_=dps, func=AF.Exp,
                                         bias=dtb(md))
            nc.sync.dma_start(out=dbl_d[:, :], in_=dblh)

            # B/C partition-broadcasts: emit right away so the SP queue
            # streams them while dt/softplus still computes
            bbt, cbt = [], []
            for n in range(NS):
                bbn = sp.tile([128, L], BF16, tag="bb", bufs=5, name="bb")
                nc.sync.dma_start(
                    out=bbn, in_=dbl_d[n:n + 1, :].to_broadcast([128, L]))
                cbn = sp.tile([128, L], BF16, tag="cbr", bufs=5, name="cbr")
                nc.sync.dma_start(
                    out=cbn,
                    in_=dbl_d[NS + n:NS + n + 1, :].to_broadcast([128, L]))
                bbt.append(bbn)
                cbt.append(cbn)

            dview = delta.rearrange("p (c t) -> p c t", c=2)

            # ---- z part: sz = z / (1+exp(-z)) ----
            sz = []
            for mz in range(2):
                z_t = ap.tile([128, L], BF16, tag=f"z{mz}", name="z_t")
                sz_t = ap.tile([128, L], BF16, tag=f"sz{mz}", name="sz_t")
                for nh in range(2):
                    sl = slice(nh * 512, (nh + 1) * 512)
                    ps = pp.tile([128, 512], F32, tag="mm", bufs=2,
                                 name="zps")
                    for k in range(2):
                        nc.tensor.matmul(
                            ps, wz(k)[:, mz * 128:(mz + 1) * 128],
                            xn[k][:, sl], start=(k == 0), stop=(k == 1))
                    nc.vector.tensor_scalar_mul(out=z_t[:, sl], in0=ps,
                                                scalar1=1.0)
                    sg = ap.tile([128, 512], BF16, tag="sg", bufs=4,
                                 name="sg")
                    nc.scalar.activation(out=sg, in_=ps, func=AF.Sigmoid)
                    nc.gpsimd.tensor_mul(out=sz_t[:, sl],
                                          in0=z_t[:, sl], in1=sg)
                sz.append(sz_t)


            # deferred softplus Ln: delta = ln(1 + e^x), per md half
            for md in range(2):
                dsl = delta[:, md * L:(md + 1) * L]
                nc.scalar.activation(out=dsl, in_=dsl, func=AF.Ln,
                                     bias=onesb)

            # ---- du = delta * u ; ud = u * D ----
            du = ap.tile([128, 2 * L], BF16, tag="du", name="du")
            duv = du.rearrange("p (c t) -> p c t", c=2)
            ud = []
            for md in range(2):
                nc.gpsimd.tensor_mul(out=duv[:, md], in0=dview[:, md],
                                      in1=ut[md])
                ud_t = ap.tile([128, L], BF16, tag=f"ud{md}", name="ud_t")
                nc.gpsimd.tensor_scalar_mul(out=ud_t, in0=ut[md],
                                            scalar1=dcol(md))
                ud.append(ud_t)

            # ---- selective scan; y accumulated in PSUM via PE ----
            yacc = [pp.tile([128, L], F32, tag=f"yacc{md}", name="yacc")
                    for md in range(2)]
            da_keep = {}
            for n in range(NS):
                if n in (6, 7):
                    da = sp.tile([128, 2 * L], BF16, tag=f"dakeep{n}",
                                 bufs=1, name="da")
                else:
                    da = sp.tile([128, 2 * L], BF16, tag="da", bufs=3,
                                 name="da")
                if n in _PROD:
                    ja, jb = _PROD[n]
                    nc.vector.tensor_mul(out=da, in0=da_keep[ja],
                                         in1=da_keep[jb])
                else:
                    nc.scalar.activation(out=da, in_=delta, func=AF.Exp,
                                         scale=anscale(n))
                if n in (6, 7):
                    da_keep[n] = da
                dav = da.rearrange("p (c t) -> p c t", c=2)

                dbu = sp.tile([128, 2 * L], BF16, tag="dbu", bufs=3,
                              name="dbu")
                dbuv = dbu.rearrange("p (c t) -> p c t", c=2)
                for md in range(2):
                    nc.gpsimd.tensor_mul(out=dbuv[:, md], in0=duv[:, md],
                                         in1=bbt[n])

                h = sp.tile([128, 2 * L], BF16, tag="h", bufs=3, name="h")
                hv = h.rearrange("p (c t) -> p c t", c=2)
                for md in range(2):
                    nc.vector.tensor_tensor_scan(
                        out=hv[:, md], data0=dav[:, md], data1=dbuv[:, md],
                        initial=0.0, op0=OP.mult, op1=OP.add)

                yt = sp.tile([128, 2 * L], BF16, tag="yt", bufs=2, name="yt")
                ytv = yt.rearrange("p (c t) -> p c t", c=2)
                eng0 = nc.gpsimd if n in (3, 7, 11, 15) else nc.vector
                eng0.tensor_mul(out=ytv[:, 0], in0=hv[:, 0], in1=cbt[n])
                nc.gpsimd.tensor_mul(out=ytv[:, 1], in0=hv[:, 1],
                                     in1=cbt[n])
                for md in range(2):
                    for q in range(2):
                        sl = slice(q * 512, (q + 1) * 512)
                        nc.tensor.matmul(
                            yacc[md][:, sl], ident, ytv[:, md][:, sl],
                            start=(n == 0), stop=(n == NS - 1))

            # ---- tail: y = (yacc + u*D) * silu(z); partial out-proj ----
            yf = []
            for md in range(2):
                y1 = ap.tile([128, L], BF16, tag=f"y1{md}", name="y1")
                nc.vector.tensor_add(out=y1, in0=yacc[md], in1=ud[md])
                yf_t = ap.tile([128, L], BF16, tag=f"yf{md}", name="yf_t")
                nc.gpsimd.tensor_mul(out=yf_t, in0=y1, in1=sz[md])
                yf.append(yf_t)
            for mc in range(2):
                pt = ap.tile([128, L], BF16, tag="part", bufs=2, name="pt")
                for nh in range(2):
                    sl = slice(nh * 512, (nh + 1) * 512)
                    po = pp.tile([128, 512], F32, tag="mm", bufs=2,
                                 name="po")
                    for k in range(2):
                        nc.tensor.matmul(
                            po, ow(k)[:, mc * 128:(mc + 1) * 128],
                            yf[k][:, sl], start=(k == 0), stop=(k == 1))
                    nc.scalar.activation(out=pt[:, sl], in_=po,
                                         func=AF.Copy)
                nc.sync.dma_start(
                    out=part[mc * 128:(mc + 1) * 128, :], in_=pt)
    nc.compile()
    return nc


def make_in_maps(inputs):
    x = np.asarray(inputs["x"], np.float32)
    maps = []
    for c in range(NCHIP):
        dr, b, half = c // 4, (c // 2) % 2, c % 2
        p = "f_" if dr == 0 else "b_"
        in_w = np.asarray(inputs[p + "in_w"], np.float32)
        convw = np.asarray(inputs[p + "conv_w"], np.float32)
        convb = np.asarray(inputs[p + "conv_b"], np.float32)
        xpj = np.asarray(inputs[p + "xproj_w"], np.float32)
        dtw = np.asarray(inputs[p + "dt_w"], np.float32)
        dtb = np.asarray(inputs[p + "dt_b"], np.float32)
        dpar = np.asarray(inputs[p + "D"], np.float32)
        outw = np.asarray(inputs["out_w"], np.float32)
        ln_g = np.asarray(inputs["ln_g"], np.float32)
        ln_b = np.asarray(inputs["ln_b"], np.float32)

        px = np.concatenate([np.arange(DL) + half * DL,
                             np.arange(DL) + (1 - half) * DL])
        loc = px[:DL]
        xin = x[b] if dr == 0 else x[b, :, ::-1, ::-1]

        w16 = np.zeros((128, _NB16), np.float32)
        wxT = in_w[:DF][px].T          # [C, DF]
        w16[:, _WX:_WX + 512] = wxT[0:128]
        w16[:, _WX + 512:_WX + 1024] = wxT[128:256]
        wzT = in_w[DF:][loc].T         # [C, DL]
        w16[:, _WZ:_WZ + 256] = wzT[0:128]
        w16[:, _WZ + 256:_WZ + 512] = wzT[128:256]
        xpwT = xpj[:, px].T            # [DF, 48]
        for k in range(4):
            w16[:, _XPW + k * 48:_XPW + (k + 1) * 48] = \
                xpwT[k * 128:(k + 1) * 128]
        w16[0:RK, _DTW:_DTW + 256] = dtw[loc].T
        owT = outw[:, loc].T           # [DL, C]
        w16[:, _OW:_OW + 256] = owT[0:128]
        w16[:, _OW + 256:_OW + 512] = owT[128:256]
        w16[:, _ID:_ID + 128] = np.eye(128, dtype=np.float32)
        w16[0, _ONES1:_ONES1 + 128] = 1.0
        cwl = convw[:, 0, :][px]       # [DF, 4]
        cbl = convb[px]
        for m in range(4):
            for k in range(4):
                off = _CD + (m * 4 + k) * 128
                w16[:, off:off + 128] = np.diag(
                    cwl[m * 128:(m + 1) * 128, k])
            off = _CBD + m * 128
            w16[:, off:off + 128] = np.diag(cbl[m * 128:(m + 1) * 128])

        w32 = np.zeros((128, _NF32), np.float32)
        for md in range(2):
            w32[:, _DTB + md] = dtb[loc][md * 128:(md + 1) * 128]
            w32[:, _DP + md] = dpar[loc][md * 128:(md + 1) * 128]
        for g in range(2):
            w32[:, _G4 + g] = 0.25 * ln_g[g * 128:(g + 1) * 128]
            w32[:, _GN + g] = -ln_g[g * 128:(g + 1) * 128]
            w32[:, _BC + g] = ln_b[g * 128:(g + 1) * 128]
        for n in range(NS):
            w32[:, _AN + n] = -float(n + 1)
        for m in range(4):
            w32[:, _CB + m] = cbl[m * 128:(m + 1) * 128]
            w32[:, _CBN + m] = -cbl[m * 128:(m + 1) * 128]

        maps.append({
            "xin": np.ascontiguousarray(xin.reshape(C, 64 * 64)),
            "wb16": w16.astype(BF16NP),
            "wf32": np.ascontiguousarray(w32),
        })
    return maps


def combine(parts, x):
    out = np.empty_like(x)
    for b in range(2):
        acc = np.zeros((C, L), np.float32)
        for c in range(NCHIP):
            dr, bb, _ = c // 4, (c // 2) % 2, c % 2
            if bb != b:
                continue
            pc = np.asarray(parts[c], np.float32)
            if dr == 1:
                pc = pc[:, ::-1]
            acc += pc
        o = acc.reshape(C, 32, 32)
        o = np.repeat(np.repeat(o, 2, axis=1), 2, axis=2)
        out[b] = o + x[b]
    return out


_NC_CACHE = None


def _get_nc():
    global _NC_CACHE
    if _NC_CACHE is None:
        _NC_CACHE = build_nc()
    return _NC_CACHE


def kernel(**inputs):
    from concourse.bass_utils import run_bass_kernel_spmd

    nc = _get_nc()
    in_maps = make_in_maps(inputs)
    res = run_bass_kernel_spmd(nc, in_maps, core_ids=list(range(NCHIP)))
    parts = [r["part"] for r in res.results]
    return combine(parts, np.asarray(inputs["x"], np.float32))
